# revision 38
# baseline (speedup 1.0000x reference)
"""Trainium2 Bass kernel for nn_ActorSlowInParallel.

The reference computes, for x = [obs | hidden0] ([B, 5653]):
    new_x      = x @ W_mean.T + b_mean          [B, 5640]
    new_hidden = relu(new_x[:, :5632])
    new_mean   = new_x[:, 5632:]
    new_lstd   = x @ W_logstd.T + b_logstd      [B, 8]
    log_std    = -5 + 3.5 * (tanh(prev_logstd) + 1)
returns (prev_mean, log_std, new_hidden, new_mean, new_lstd).

W_mean is block-banded (12 staircase blocks): block 0 maps obs[21] ->
rows 0:512, blocks 1..10 map hidden0 chunk (i-1) -> rows 512i:512i+512,
block 11 maps hidden0 chunk 10 -> rows 5632:5640.  Only ~8% of the dense
matrix is nonzero, so we do 12 block matmuls instead of one dense one.

Distribution: data-parallel over batch.  Each of the 8 cores gets 1024
batch rows; weights are replicated.  No collectives.  Activations are
fed feature-major ([in, batch], transposed on host) so the contraction
dim lands on SBUF partitions; outputs are produced feature-major and
transposed back on host.
"""

import os
import sys

sys.path.insert(0, "/opt/trn_rl_repo")

import numpy as np

# ---------------------------------------------------------------- consts
P = 128
B = 8192
NCORES = 8
BC = B // NCORES            # 1024 batch rows per core
OBS_D = 21
HID = 512
OVERALL = 5632              # 11 * 512
IN_DIM = 5653
OUT_DIM = 5640
ACT_D = 8
NBLK = 11                   # h0-consuming blocks (1..11); block 11 is the mean head
NT = BC // 512              # moving tiles per batch (2)
NJ = BC // P                # 128-wide batch tiles (8)

_KDT = os.environ.get("K_DTYPE", "bf16")

_cache = {}


# ------------------------------------------------------- walrus workaround
def _patch_drain_waits():
    """The nix walrus build rejects >1 sync wait per instruction.  Tile
    attaches one wait per producer processor.  Spill the excess onto
    same-engine nops emitted directly before each instruction."""
    from concourse import tile, mybir

    if getattr(tile.TileContext, "_drain_waits_patched", False):
        return
    orig = tile.TileContext._drain_and_barrier
    MAXW = 1

    orig_lower = tile.TileContext._lower_ordered_insts

    def patched_lower(self, ordered):
        nc = self.nc
        for bb_name in list(ordered.keys()):
            new = []
            for inst in ordered[bb_name]:
                si = getattr(inst, "sync_info", None)
                if si is not None and si.on_wait and len(si.on_wait) > MAXW:
                    waits = list(si.on_wait)
                    extra, keep = waits[:-MAXW], waits[-MAXW:]
                    for i in range(0, len(extra), MAXW):
                        nop = mybir.InstNoOp(
                            name=nc.get_next_instruction_name(),
                            sync_info=mybir.SyncInfo(
                                on_wait=extra[i:i + MAXW], on_update=[]
                            ),
                            bass_nofuse=True,
                            engine=inst.engine,
                        )
                        new.append(nop)
                    inst.sync_info = mybir.SyncInfo(
                        on_wait=keep, on_update=list(si.on_update)
                    )
                new.append(inst)
            ordered[bb_name] = new
        return orig_lower(self, ordered)

    tile.TileContext._lower_ordered_insts = patched_lower

    def patched(self, tick_clock, wait_clock):
        nc = self.nc
        spill = [nc.sync.nop(nofuse=True) for _ in range(32)]
        orig(self, tick_clock, wait_clock)
        bb = None
        for func in nc.m.functions:
            for block in func.blocks:
                if any(i.name == spill[0].ins.name for i in block.instructions):
                    bb = block
                    break
        assert bb is not None
        drain = None
        seen = False
        for ins in bb.instructions:
            if ins.name == spill[0].ins.name:
                seen = True
            if seen and isinstance(ins, mybir.InstDrain):
                drain = ins
                break
        assert drain is not None
        waits = list(drain.sync_info.on_wait) if drain.sync_info else []
        if len(waits) > MAXW:
            excess = waits[MAXW:]
            drain.sync_info = mybir.SyncInfo(
                on_wait=waits[:MAXW], on_update=list(drain.sync_info.on_update)
            )
            assert len(excess) <= len(spill) * MAXW
            for i, nop in enumerate(spill):
                chunk = excess[i * MAXW:(i + 1) * MAXW]
                if not chunk:
                    break
                nop.ins.sync_info = mybir.SyncInfo(on_wait=chunk, on_update=[])

    tile.TileContext._drain_and_barrier = patched
    tile.TileContext._drain_waits_patched = True


# ------------------------------------------------------------- device code
def _build_nc():
    import concourse.bass as bass
    import concourse.mybir as mybir
    from concourse.tile import TileContext
    from contextlib import ExitStack

    _patch_drain_waits()

    f32 = mybir.dt.float32
    Alu = mybir.AluOpType
    Af = mybir.ActivationFunctionType

    mdt = {"bf16": mybir.dt.bfloat16, "f32r": mybir.dt.float32r,
           "f32": f32}[_KDT]

    def mm_ap(ap):
        return ap

    nc = bass.Bass()

    obsT = nc.declare_dram_parameter("obsT", [OBS_D, BC], mdt, isOutput=False)
    # partition-major: h0pm[p, 1024*k + n] = hidden0_shard[n, 128*k + p]
    h0pm = nc.declare_dram_parameter("h0pm", [P, 44 * BC], mdt, isOutput=False)
    wt0 = nc.declare_dram_parameter("wt0", [OBS_D, HID], mdt, isOutput=False)
    # partition-major: wtmpm[p, 512*q + m] = W_block(q//4).T[128*(q%4) + p, m]
    wtmpm = nc.declare_dram_parameter("wtmpm", [P, 40 * HID], mdt, isOutput=False)
    wt11 = nc.declare_dram_parameter("wt11", [HID, ACT_D], mdt, isOutput=False)
    wls = nc.declare_dram_parameter("wls", [IN_DIM, ACT_D], mdt, isOutput=False)
    bm = nc.declare_dram_parameter("bm", [45 * P, 1], f32, isOutput=False)
    blsb = nc.declare_dram_parameter("blsb", [ACT_D, 1], f32, isOutput=False)
    pls = nc.declare_dram_parameter("pls", [P, 64], f32, isOutput=False)

    # partition-major output: out_pm[p, 1024*t + n] = new_x.T[128*t + p, n]
    # (t = 0..43 hidden m-tiles; t = 44: partitions 0:8 new_mean, 8:16 new_lstd)
    out_pm = nc.declare_dram_parameter("out_pm", [P, 45 * BC], mdt, isOutput=True)
    out_sm = nc.declare_dram_parameter("out_sm", [P, 64], f32, isOutput=True)

    with TileContext(nc) as tc, ExitStack() as ctx:
        misc = ctx.enter_context(tc.tile_pool(name="misc", bufs=1))
        xp = ctx.enter_context(tc.tile_pool(name="xp", bufs=4))
        outp = ctx.enter_context(tc.tile_pool(name="outp", bufs=5))
        psum = ctx.enter_context(tc.tile_pool(name="psum", bufs=5, space="PSUM"))
        psls = ctx.enter_context(tc.tile_pool(name="psls", bufs=1, space="PSUM"))

        # ---- input staging: per-block, chunk-granular tiles so the PE can
        # start on the first arriving 512KB and deps stay fine-grained
        x_tiles = {}

        def load_x(i):
            if i in x_tiles or i > NBLK:
                return
            c = i - 1
            if False:
                pass
            else:
                t = xp.tile([P, 4, BC], mdt, tag="x", name=f"x{i}")
                nc.sync.dma_start(
                    out=t[:, :, :],
                    in_=h0pm[:, 4 * BC * c:4 * BC * i].rearrange(
                        "p (k n) -> p k n", n=BC
                    ),
                )
                x_tiles[i] = [t[:, k, :] for k in range(4)]

        w_tiles = {}

        def load_w(i):
            if i in w_tiles or i > 10:
                return
            c = i - 1
            t = misc.tile([P, 4, HID], mdt, tag=f"wb{i}", name=f"wb{i}")
            nc.sync.dma_start(
                out=t[:, :, :],
                in_=wtmpm[:, 4 * HID * c:4 * HID * i].rearrange(
                    "p (k m) -> p k m", m=HID
                ),
            )
            w_tiles[i] = [t[:, k, :] for k in range(4)]

        load_x(1)
        load_w(1)

        def x_of(i):
            return x_tiles.pop(i)

        def w_of(i):
            return w_tiles.pop(i)

        # ---- resident small tensors
        obs_sb = misc.tile([P, BC], mdt, tag="obs")
        nc.sync.dma_start(out=obs_sb[0:OBS_D, :], in_=obsT[:, :])
        wt0_sb = misc.tile([P, HID], mdt, tag="wt0")
        nc.sync.dma_start(out=wt0_sb[0:OBS_D, :], in_=wt0[:, :])
        wt11_sb = misc.tile([P, 4, ACT_D], mdt, tag="wt11")
        nc.sync.dma_start(
            out=wt11_sb[:, :, :],
            in_=wt11[:, :].rearrange("(k p) e -> p k e", p=P),
        )
        wls_obs = misc.tile([P, ACT_D], mdt, tag="wlso")
        nc.sync.dma_start(out=wls_obs[0:OBS_D, :], in_=wls[0:OBS_D, :])
        wls_h0 = misc.tile([P, 44, ACT_D], mdt, tag="wlsh")
        nc.sync.dma_start(
            out=wls_h0[:, :, :],
            in_=wls[OBS_D:, :].rearrange("(k p) e -> p k e", p=P),
        )
        bm_sb = misc.tile([P, 45, 1], f32, tag="bm")
        nc.sync.dma_start(
            out=bm_sb[:, :, :], in_=bm[:, :].rearrange("(t p) o -> p t o", p=P)
        )
        blsb_sb = misc.tile([P, 1], f32, tag="blsb")
        nc.sync.dma_start(out=blsb_sb[0:ACT_D, :], in_=blsb[:, :])
        pls_sb = misc.tile([P, 64], f32, tag="pls")
        nc.sync.dma_start(out=pls_sb[:, :], in_=pls[:, :])
        sm_sb = misc.tile([P, 64], f32, tag="sm")
        tanh_sb = misc.tile([P, 64], f32, tag="tanh")

        # ---- log_std transform of prev_logstd (elementwise)
        nc.scalar.activation(tanh_sb[:, :], pls_sb[:, :], Af.Tanh)
        nc.vector.tensor_scalar(
            out=sm_sb[:, :], in0=tanh_sb[:, :],
            scalar1=3.5, scalar2=-1.5, op0=Alu.mult, op1=Alu.add,
        )
        nc.sync.dma_start(out=out_sm[:, :], in_=sm_sb[:, :])

        # ---- logstd accumulators: [8, 512] per batch half x 4 column strips.
        # Zero each bank with a dummy start=True matmul covering all 128
        # partitions so every strip can then accumulate with start=False
        # (strip-local start=True would poison the other strips' zero region).
        ls_ps = [psls.tile([P, 512], f32, tag=f"lsps{n}", name=f"lsps{n}") for n in range(NT)]
        zmm = misc.tile([1, 512], mdt, tag="zmm")
        nc.vector.memset(zmm[:, :], 0.0)
        for n in range(NT):
            nc.tensor.matmul(
                ls_ps[n][0:P, :],
                mm_ap(zmm[0:1, 0:P]),
                mm_ap(zmm[0:1, 0:512]),
                start=True, stop=False,
            )

        # obs chunk of the logstd matmul (K=21), column strip 0
        for n in range(NT):
            nc.tensor.matmul(
                ls_ps[n][0:ACT_D, :],
                mm_ap(wls_obs[0:OBS_D, :]),
                mm_ap(obs_sb[0:OBS_D, 512 * n:512 * (n + 1)]),
                start=False, stop=False,
                tile_position=(0, 0),
            )

        ep_cnt = [0]

        def epilogue_half(ps_ap, dst_row, bias, relu, n):
            """psum [rows,512] + bias (per-partition) [+ relu] -> sbuf half."""
            rows = ps_ap.partition_size()
            ep_cnt[0] += 1
            dst = dst_row[0:rows, 512 * n:512 * (n + 1)]
            if relu:
                if ep_cnt[0] % 2 == 0:
                    nc.vector.tensor_scalar(
                        out=dst, in0=ps_ap, scalar1=bias,
                        scalar2=0.0, op0=Alu.add, op1=Alu.max,
                    )
                else:
                    nc.scalar.activation(dst, ps_ap, Af.Relu, bias=bias)
            else:
                nc.vector.tensor_scalar(
                    out=dst, in0=ps_ap, scalar1=bias,
                    scalar2=None, op0=Alu.add,
                )

        # ---- block 0: obs -> out rows 0:512
        o_sb0 = outp.tile([P, 4, BC], mdt, tag="osb", name="osb0")
        for m in range(4):
            for n in range(NT):
                ps = psum.tile([P, 512], f32, tag="ps")
                nc.tensor.matmul(
                    ps[:, :],
                    mm_ap(wt0_sb[0:OBS_D, P * m:P * (m + 1)]),
                    mm_ap(obs_sb[0:OBS_D, 512 * n:512 * (n + 1)]),
                    start=True, stop=True,
                )
                epilogue_half(ps[:, :], o_sb0[:, m, :], bm_sb[0:P, m, 0:1], True, n)
        nc.gpsimd.dma_start(
            out=out_pm[:, 0:4 * BC].rearrange("p (m n) -> p m n", n=BC),
            in_=o_sb0[:, :, :],
        )

        # ---- blocks 1..11 over hidden0 chunks
        for i in range(1, NBLK + 1):
            c = i - 1  # h0 chunk index
            xt = x_of(i)
            if i <= 10:
                w3 = w_of(i)
                o_sb = outp.tile([P, 4, BC], mdt, tag="osb", name=f"osb{i}")
                if False:
                    pass
                else:
                    for m in range(4):
                        pss = [psum.tile([P, 512], f32, tag="ps", name=f"ps{i}_{m}_{n2}") for n2 in range(NT)]
                        for k in range(4):
                            for n in range(NT):
                                nc.tensor.matmul(
                                    pss[n][:, :],
                                    mm_ap(w3[k][:, P * m:P * (m + 1)]),
                                    mm_ap(xt[k][:, 512 * n:512 * (n + 1)]),
                                    start=(k == 0), stop=(k == 3),
                                )
                        for n in range(NT):
                            epilogue_half(
                                pss[n][:, :], o_sb[:, m, :],
                                bm_sb[0:P, 4 * i + m, 0:1], True, n,
                            )
                nc.gpsimd.dma_start(
                    out=out_pm[:, 4 * BC * i:4 * BC * (i + 1)].rearrange(
                        "p (m n) -> p m n", n=BC
                    ),
                    in_=o_sb[:, :, :],
                )
            else:
                # mean head: 8 output rows, no relu
                o_sb = outp.tile([P, BC], mdt, tag="osbm", name="osbm")
                pss = [psum.tile([P, 512], f32, tag="ps", name=f"psmh_{n2}") for n2 in range(NT)]
                for k in range(4):
                    for n in range(NT):
                        nc.tensor.matmul(
                            pss[n][0:ACT_D, :],
                            mm_ap(wt11_sb[:, k, :]),
                            mm_ap(xt[k][:, 512 * n:512 * (n + 1)]),
                            start=(k == 0), stop=(k == 3),
                        )
                for n in range(NT):
                    epilogue_half(
                        pss[n][0:ACT_D, :], o_sb, bm_sb[0:ACT_D, 44, 0:1], False, n
                    )
                nc.gpsimd.dma_start(
                    out=out_pm[0:ACT_D, 44 * BC:45 * BC], in_=o_sb[0:ACT_D, :]
                )

            # logstd accumulation: strip k of the PE array accumulates every
            # (4c+k)-th chunk at psum partitions 32k:32k+8 -> 4 concurrent
            # M=8 matmuls via column tiling
            for n in range(NT):
                for k in range(4):
                    nc.tensor.matmul(
                        ls_ps[n][32 * k:32 * k + ACT_D, :],
                        mm_ap(wls_h0[:, 4 * c + k, :]),
                        mm_ap(xt[k][:, 512 * n:512 * (n + 1)]),
                        start=False,
                        stop=(i == NBLK),
                        tile_position=(0, 32 * k),
                    )
            load_x(i + 1)
            load_w(i + 1)
            load_x(i + 2)

        # ---- new_log_std epilogue: reduce the 4 strips, add bias
        ols = outp.tile([P, BC], mdt, tag="osbm", name="ols")
        for n in range(NT):
            red = misc.tile([P, 4, 512], f32, tag=f"red{n}", name=f"red{n}")
            for g in range(4):
                nc.vector.tensor_copy(
                    out=red[32 * g:32 * g + ACT_D, g, :],
                    in_=ls_ps[n][32 * g:32 * g + ACT_D, :],
                )
            for g in range(1, 4):
                nc.sync.dma_start(
                    out=red[0:ACT_D, g, :], in_=red[32 * g:32 * g + ACT_D, g, :]
                )
            nc.vector.tensor_tensor(
                out=red[0:ACT_D, 0, :], in0=red[0:ACT_D, 0, :],
                in1=red[0:ACT_D, 1, :], op=Alu.add,
            )
            nc.vector.tensor_tensor(
                out=red[0:ACT_D, 2, :], in0=red[0:ACT_D, 2, :],
                in1=red[0:ACT_D, 3, :], op=Alu.add,
            )
            nc.vector.tensor_tensor(
                out=red[0:ACT_D, 0, :], in0=red[0:ACT_D, 0, :],
                in1=red[0:ACT_D, 2, :], op=Alu.add,
            )
            nc.vector.tensor_scalar(
                out=ols[0:ACT_D, 512 * n:512 * (n + 1)], in0=red[0:ACT_D, 0, :],
                scalar1=blsb_sb[0:ACT_D, 0:1], scalar2=None, op0=Alu.add,
            )
        nc.gpsimd.dma_start(
            out=out_pm[ACT_D:2 * ACT_D, 44 * BC:45 * BC], in_=ols[0:ACT_D, :]
        )

    return nc


def _get_nc():
    if "nc" not in _cache:
        _cache["nc"] = _build_nc()
    return _cache["nc"]


# ---------------------------------------------------------------- host code
def kernel(obs, hidden0, prev_mean, prev_logstd, W_mean, b_mean,
           W_logstd, b_logstd):
    from concourse.bass_utils import run_bass_kernel_spmd
    import ml_dtypes

    mnp = {"bf16": ml_dtypes.bfloat16, "f32r": np.float32,
           "f32": np.float32}[_KDT]

    obs = np.asarray(obs, np.float32)
    h0 = np.asarray(hidden0, np.float32)
    prev_mean = np.asarray(prev_mean, np.float32)
    prev_logstd = np.asarray(prev_logstd, np.float32)
    Wm = np.asarray(W_mean, np.float32)
    b_mean = np.asarray(b_mean, np.float32)
    Wls = np.asarray(W_logstd, np.float32)
    b_logstd = np.asarray(b_logstd, np.float32)

    # shared (replicated) weight prep
    wt0 = np.ascontiguousarray(Wm[0:HID, 0:OBS_D].T).astype(mnp)      # [21, 512]
    wtm = np.concatenate(
        [
            np.ascontiguousarray(
                Wm[HID * i:HID * (i + 1),
                   OBS_D + HID * (i - 1):OBS_D + HID * i].T
            )
            for i in range(1, 11)
        ],
        axis=0,
    ).astype(mnp)                                                      # [5120, 512]
    # partition-major: [128, 40*512]
    wtmpm = np.ascontiguousarray(
        wtm.reshape(40, P, HID).transpose(1, 0, 2).reshape(P, 40 * HID)
    )
    wt11 = np.ascontiguousarray(Wm[OVERALL:OUT_DIM, IN_DIM - HID:IN_DIM].T).astype(mnp)
    wls = np.ascontiguousarray(Wls.T).astype(mnp)                      # [5653, 8]
    bm = np.zeros((45 * P, 1), np.float32)
    bm[:OUT_DIM, 0] = b_mean
    blsb = np.ascontiguousarray(b_logstd.reshape(ACT_D, 1), np.float32)

    in_maps = []
    for c in range(NCORES):
        sl = slice(c * BC, (c + 1) * BC)
        in_maps.append({
            "obsT": np.ascontiguousarray(obs[sl].T).astype(mnp),
            "h0pm": np.ascontiguousarray(
                h0[sl].T.reshape(44, P, BC).transpose(1, 0, 2)
            ).reshape(P, 44 * BC).astype(mnp),
            "wt0": wt0, "wtmpm": wtmpm, "wt11": wt11, "wls": wls,
            "bm": bm, "blsb": blsb,
            "pls": np.ascontiguousarray(prev_logstd[sl]).reshape(P, 64),
        })

    nc = _get_nc()
    _cache["last_in_maps"] = in_maps
    res = run_bass_kernel_spmd(nc, in_maps, core_ids=list(range(NCORES)))

    new_hidden = np.empty((B, OVERALL), np.float32)
    new_mean = np.empty((B, ACT_D), np.float32)
    new_lstd = np.empty((B, ACT_D), np.float32)
    log_std = np.empty((B, ACT_D), np.float32)
    for c in range(NCORES):
        sl = slice(c * BC, (c + 1) * BC)
        o = res.results[c]["out_pm"].astype(np.float32).reshape(P, 45, BC)
        # feature f = 128*t + p lives at o[p, t, n]
        new_hidden[sl] = o[:, 0:44, :].transpose(2, 1, 0).reshape(BC, OVERALL)
        new_mean[sl] = o[0:ACT_D, 44, :].T
        new_lstd[sl] = o[ACT_D:2 * ACT_D, 44, :].T
        sm = res.results[c]["out_sm"]
        log_std[sl] = sm.reshape(BC, ACT_D)

    return (prev_mean, log_std, new_hidden, new_mean, new_lstd)


# revision 39
# speedup vs baseline: 1.1750x; 1.1750x over previous
"""Trainium2 Bass kernel for nn_ActorSlowInParallel.

The reference computes, for x = [obs | hidden0] ([B, 5653]):
    new_x      = x @ W_mean.T + b_mean          [B, 5640]
    new_hidden = relu(new_x[:, :5632])
    new_mean   = new_x[:, 5632:]
    new_lstd   = x @ W_logstd.T + b_logstd      [B, 8]
    log_std    = -5 + 3.5 * (tanh(prev_logstd) + 1)
returns (prev_mean, log_std, new_hidden, new_mean, new_lstd).

W_mean is block-banded (12 staircase blocks): block 0 maps obs[21] ->
rows 0:512, blocks 1..10 map hidden0 chunk (i-1) -> rows 512i:512i+512,
block 11 maps hidden0 chunk 10 -> rows 5632:5640.  Only ~8% of the dense
matrix is nonzero, so we do 12 block matmuls instead of one dense one.

Distribution: data-parallel over batch.  Each of the 8 cores gets 1024
batch rows; weights are replicated.  No collectives.  Activations are
fed feature-major ([in, batch], transposed on host) so the contraction
dim lands on SBUF partitions; outputs are produced feature-major and
transposed back on host.
"""

import os
import sys

sys.path.insert(0, "/opt/trn_rl_repo")

import numpy as np

# ---------------------------------------------------------------- consts
P = 128
B = 8192
NCORES = 8
BC = B // NCORES            # 1024 batch rows per core
OBS_D = 21
HID = 512
OVERALL = 5632              # 11 * 512
IN_DIM = 5653
OUT_DIM = 5640
ACT_D = 8
NBLK = 11                   # h0-consuming blocks (1..11); block 11 is the mean head
NT = BC // 512              # moving tiles per batch (2)
NJ = BC // P                # 128-wide batch tiles (8)

_KDT = os.environ.get("K_DTYPE", "bf16")

_cache = {}


# ------------------------------------------------------- walrus workaround
def _patch_drain_waits():
    """The nix walrus build rejects >1 sync wait per instruction.  Tile
    attaches one wait per producer processor.  Spill the excess onto
    same-engine nops emitted directly before each instruction."""
    from concourse import tile, mybir

    if getattr(tile.TileContext, "_drain_waits_patched", False):
        return
    orig = tile.TileContext._drain_and_barrier
    MAXW = 1

    orig_lower = tile.TileContext._lower_ordered_insts

    def patched_lower(self, ordered):
        nc = self.nc
        for bb_name in list(ordered.keys()):
            new = []
            for inst in ordered[bb_name]:
                si = getattr(inst, "sync_info", None)
                if si is not None and si.on_wait and len(si.on_wait) > MAXW:
                    waits = list(si.on_wait)
                    extra, keep = waits[:-MAXW], waits[-MAXW:]
                    for i in range(0, len(extra), MAXW):
                        nop = mybir.InstNoOp(
                            name=nc.get_next_instruction_name(),
                            sync_info=mybir.SyncInfo(
                                on_wait=extra[i:i + MAXW], on_update=[]
                            ),
                            bass_nofuse=True,
                            engine=inst.engine,
                        )
                        new.append(nop)
                    inst.sync_info = mybir.SyncInfo(
                        on_wait=keep, on_update=list(si.on_update)
                    )
                new.append(inst)
            ordered[bb_name] = new
        return orig_lower(self, ordered)

    tile.TileContext._lower_ordered_insts = patched_lower

    def patched(self, tick_clock, wait_clock):
        nc = self.nc
        spill = [nc.sync.nop(nofuse=True) for _ in range(32)]
        orig(self, tick_clock, wait_clock)
        bb = None
        for func in nc.m.functions:
            for block in func.blocks:
                if any(i.name == spill[0].ins.name for i in block.instructions):
                    bb = block
                    break
        assert bb is not None
        drain = None
        seen = False
        for ins in bb.instructions:
            if ins.name == spill[0].ins.name:
                seen = True
            if seen and isinstance(ins, mybir.InstDrain):
                drain = ins
                break
        assert drain is not None
        waits = list(drain.sync_info.on_wait) if drain.sync_info else []
        if len(waits) > MAXW:
            excess = waits[MAXW:]
            drain.sync_info = mybir.SyncInfo(
                on_wait=waits[:MAXW], on_update=list(drain.sync_info.on_update)
            )
            assert len(excess) <= len(spill) * MAXW
            for i, nop in enumerate(spill):
                chunk = excess[i * MAXW:(i + 1) * MAXW]
                if not chunk:
                    break
                nop.ins.sync_info = mybir.SyncInfo(on_wait=chunk, on_update=[])

    tile.TileContext._drain_and_barrier = patched
    tile.TileContext._drain_waits_patched = True


# ------------------------------------------------------------- device code
def _build_nc():
    import concourse.bass as bass
    import concourse.mybir as mybir
    from concourse.tile import TileContext
    from contextlib import ExitStack

    _patch_drain_waits()

    f32 = mybir.dt.float32
    Alu = mybir.AluOpType
    Af = mybir.ActivationFunctionType

    mdt = {"bf16": mybir.dt.bfloat16, "f32r": mybir.dt.float32r,
           "f32": f32}[_KDT]

    def mm_ap(ap):
        return ap

    nc = bass.Bass()

    obsT = nc.declare_dram_parameter("obsT", [OBS_D, BC], mdt, isOutput=False)
    # partition-major: h0pm[p, 1024*k + n] = hidden0_shard[n, 128*k + p]
    h0pm = nc.declare_dram_parameter("h0pm", [P, 44 * BC], mdt, isOutput=False)
    wt0 = nc.declare_dram_parameter("wt0", [OBS_D, HID], mdt, isOutput=False)
    # partition-major: wtmpm[p, 512*q + m] = W_block(q//4).T[128*(q%4) + p, m]
    wtmpm = nc.declare_dram_parameter("wtmpm", [P, 40 * HID], mdt, isOutput=False)
    wt11 = nc.declare_dram_parameter("wt11", [HID, ACT_D], mdt, isOutput=False)
    wls = nc.declare_dram_parameter("wls", [IN_DIM, ACT_D], mdt, isOutput=False)
    bm = nc.declare_dram_parameter("bm", [45 * P, 1], f32, isOutput=False)
    blsb = nc.declare_dram_parameter("blsb", [ACT_D, 1], f32, isOutput=False)
    pls = nc.declare_dram_parameter("pls", [P, 64], f32, isOutput=False)

    # partition-major output: out_pm[p, 1024*t + n] = new_x.T[128*t + p, n]
    # (t = 0..43 hidden m-tiles; t = 44: partitions 0:8 new_mean, 8:16 new_lstd)
    out_pm = nc.declare_dram_parameter("out_pm", [P, 45 * BC], mdt, isOutput=True)
    out_sm = nc.declare_dram_parameter("out_sm", [P, 64], f32, isOutput=True)

    with TileContext(nc) as tc, ExitStack() as ctx:
        misc = ctx.enter_context(tc.tile_pool(name="misc", bufs=1))
        xp = ctx.enter_context(tc.tile_pool(name="xp", bufs=4))
        outp = ctx.enter_context(tc.tile_pool(name="outp", bufs=4))
        psum = ctx.enter_context(tc.tile_pool(name="psum", bufs=5, space="PSUM"))
        psls = ctx.enter_context(tc.tile_pool(name="psls", bufs=1, space="PSUM"))

        # ---- input staging: per-block, chunk-granular tiles so the PE can
        # start on the first arriving 512KB and deps stay fine-grained
        x_tiles = {}

        def load_x(i):
            if i in x_tiles or i > NBLK:
                return
            c = i - 1
            if False:
                pass
            else:
                t = xp.tile([P, 4, BC], mdt, tag="x", name=f"x{i}")
                nc.sync.dma_start(
                    out=t[:, :, :],
                    in_=h0pm[:, 4 * BC * c:4 * BC * i].rearrange(
                        "p (k n) -> p k n", n=BC
                    ),
                )
                x_tiles[i] = [t[:, k, :] for k in range(4)]

        w_tiles = {}

        def load_w(i):
            if i in w_tiles or i > 10:
                return
            c = i - 1
            t = misc.tile([P, 4, HID], mdt, tag=f"wb{i}", name=f"wb{i}")
            nc.sync.dma_start(
                out=t[:, :, :],
                in_=wtmpm[:, 4 * HID * c:4 * HID * i].rearrange(
                    "p (k m) -> p k m", m=HID
                ),
            )
            w_tiles[i] = [t[:, k, :] for k in range(4)]

        load_x(1)
        load_w(1)

        def x_of(i):
            return x_tiles.pop(i)

        def w_of(i):
            return w_tiles.pop(i)

        # ---- resident small tensors
        obs_sb = misc.tile([P, BC], mdt, tag="obs")
        nc.sync.dma_start(out=obs_sb[0:OBS_D, :], in_=obsT[:, :])
        wt0_sb = misc.tile([P, HID], mdt, tag="wt0")
        nc.sync.dma_start(out=wt0_sb[0:OBS_D, :], in_=wt0[:, :])
        wt11_sb = misc.tile([P, 4, ACT_D], mdt, tag="wt11")
        nc.sync.dma_start(
            out=wt11_sb[:, :, :],
            in_=wt11[:, :].rearrange("(k p) e -> p k e", p=P),
        )
        wls_obs = misc.tile([P, ACT_D], mdt, tag="wlso")
        nc.sync.dma_start(out=wls_obs[0:OBS_D, :], in_=wls[0:OBS_D, :])
        wls_h0 = misc.tile([P, 44, ACT_D], mdt, tag="wlsh")
        nc.sync.dma_start(
            out=wls_h0[:, :, :],
            in_=wls[OBS_D:, :].rearrange("(k p) e -> p k e", p=P),
        )
        bm_sb = misc.tile([P, 45, 1], f32, tag="bm")
        nc.sync.dma_start(
            out=bm_sb[:, :, :], in_=bm[:, :].rearrange("(t p) o -> p t o", p=P)
        )
        blsb_sb = misc.tile([P, 1], f32, tag="blsb")
        nc.sync.dma_start(out=blsb_sb[0:ACT_D, :], in_=blsb[:, :])
        pls_sb = misc.tile([P, 64], f32, tag="pls")
        nc.sync.dma_start(out=pls_sb[:, :], in_=pls[:, :])
        sm_sb = misc.tile([P, 64], f32, tag="sm")
        tanh_sb = misc.tile([P, 64], f32, tag="tanh")

        # ---- log_std transform of prev_logstd (elementwise)
        nc.scalar.activation(tanh_sb[:, :], pls_sb[:, :], Af.Tanh)
        nc.vector.tensor_scalar(
            out=sm_sb[:, :], in0=tanh_sb[:, :],
            scalar1=3.5, scalar2=-1.5, op0=Alu.mult, op1=Alu.add,
        )
        nc.sync.dma_start(out=out_sm[:, :], in_=sm_sb[:, :])

        # ---- logstd accumulators: [8, 512] per batch half x 4 column strips.
        # Zero each bank with a dummy start=True matmul covering all 128
        # partitions so every strip can then accumulate with start=False
        # (strip-local start=True would poison the other strips' zero region).
        ls_ps = [psls.tile([P, 512], f32, tag=f"lsps{n}", name=f"lsps{n}") for n in range(NT)]
        zmm = misc.tile([1, 512], mdt, tag="zmm")
        nc.vector.memset(zmm[:, :], 0.0)
        for n in range(NT):
            nc.tensor.matmul(
                ls_ps[n][0:P, :],
                mm_ap(zmm[0:1, 0:P]),
                mm_ap(zmm[0:1, 0:512]),
                start=True, stop=False,
            )

        # obs chunk of the logstd matmul (K=21), column strip 0
        for n in range(NT):
            nc.tensor.matmul(
                ls_ps[n][0:ACT_D, :],
                mm_ap(wls_obs[0:OBS_D, :]),
                mm_ap(obs_sb[0:OBS_D, 512 * n:512 * (n + 1)]),
                start=False, stop=False,
                tile_position=(0, 0),
            )

        ep_cnt = [0]

        def epilogue_half(ps_ap, dst_row, bias, relu, n):
            """psum [rows,512] + bias (per-partition) [+ relu] -> sbuf half."""
            rows = ps_ap.partition_size()
            ep_cnt[0] += 1
            dst = dst_row[0:rows, 512 * n:512 * (n + 1)]
            if relu:
                if ep_cnt[0] % 2 == 0:
                    nc.vector.tensor_scalar(
                        out=dst, in0=ps_ap, scalar1=bias,
                        scalar2=0.0, op0=Alu.add, op1=Alu.max,
                    )
                else:
                    nc.scalar.activation(dst, ps_ap, Af.Relu, bias=bias)
            else:
                nc.vector.tensor_scalar(
                    out=dst, in0=ps_ap, scalar1=bias,
                    scalar2=None, op0=Alu.add,
                )

        # ---- block 0: obs -> out rows 0:512
        o_sb0 = outp.tile([P, 4, BC], mdt, tag="osb", name="osb0")
        for m in range(4):
            for n in range(NT):
                ps = psum.tile([P, 512], f32, tag="ps")
                nc.tensor.matmul(
                    ps[:, :],
                    mm_ap(wt0_sb[0:OBS_D, P * m:P * (m + 1)]),
                    mm_ap(obs_sb[0:OBS_D, 512 * n:512 * (n + 1)]),
                    start=True, stop=True,
                )
                epilogue_half(ps[:, :], o_sb0[:, m, :], bm_sb[0:P, m, 0:1], True, n)
        nc.gpsimd.dma_start(
            out=out_pm[:, 0:4 * BC].rearrange("p (m n) -> p m n", n=BC),
            in_=o_sb0[:, :, :],
        )

        # ---- blocks 1..11 over hidden0 chunks
        for i in range(1, NBLK + 1):
            c = i - 1  # h0 chunk index
            xt = x_of(i)
            if i <= 10:
                w3 = w_of(i)
                o_sb = outp.tile([P, 4, BC], mdt, tag="osb", name=f"osb{i}")
                if False:
                    pass
                else:
                    for m in range(4):
                        pss = [psum.tile([P, 512], f32, tag="ps", name=f"ps{i}_{m}_{n2}") for n2 in range(NT)]
                        for k in range(4):
                            for n in range(NT):
                                nc.tensor.matmul(
                                    pss[n][:, :],
                                    mm_ap(w3[k][:, P * m:P * (m + 1)]),
                                    mm_ap(xt[k][:, 512 * n:512 * (n + 1)]),
                                    start=(k == 0), stop=(k == 3),
                                )
                        for n in range(NT):
                            epilogue_half(
                                pss[n][:, :], o_sb[:, m, :],
                                bm_sb[0:P, 4 * i + m, 0:1], True, n,
                            )
                nc.gpsimd.dma_start(
                    out=out_pm[:, 4 * BC * i:4 * BC * (i + 1)].rearrange(
                        "p (m n) -> p m n", n=BC
                    ),
                    in_=o_sb[:, :, :],
                )
            else:
                # mean head: 8 output rows, no relu
                o_sb = outp.tile([P, BC], mdt, tag="osbm", name="osbm")
                pss = [psum.tile([P, 512], f32, tag="ps", name=f"psmh_{n2}") for n2 in range(NT)]
                for k in range(4):
                    for n in range(NT):
                        nc.tensor.matmul(
                            pss[n][0:ACT_D, :],
                            mm_ap(wt11_sb[:, k, :]),
                            mm_ap(xt[k][:, 512 * n:512 * (n + 1)]),
                            start=(k == 0), stop=(k == 3),
                        )
                for n in range(NT):
                    epilogue_half(
                        pss[n][0:ACT_D, :], o_sb, bm_sb[0:ACT_D, 44, 0:1], False, n
                    )
                nc.gpsimd.dma_start(
                    out=out_pm[0:ACT_D, 44 * BC:45 * BC], in_=o_sb[0:ACT_D, :]
                )

            # logstd accumulation: strip k of the PE array accumulates every
            # (4c+k)-th chunk at psum partitions 32k:32k+8 -> 4 concurrent
            # M=8 matmuls via column tiling
            for n in range(NT):
                for k in range(4):
                    nc.tensor.matmul(
                        ls_ps[n][32 * k:32 * k + ACT_D, :],
                        mm_ap(wls_h0[:, 4 * c + k, :]),
                        mm_ap(xt[k][:, 512 * n:512 * (n + 1)]),
                        start=False,
                        stop=(i == NBLK),
                        tile_position=(0, 32 * k),
                    )
            load_x(i + 1)
            load_w(i + 1)
            load_x(i + 2)

        # ---- new_log_std epilogue: reduce the 4 strips, add bias
        ols = outp.tile([P, BC], mdt, tag="osbm", name="ols")
        for n in range(NT):
            red = misc.tile([P, 4, 512], f32, tag=f"red{n}", name=f"red{n}")
            for g in range(4):
                nc.vector.tensor_copy(
                    out=red[32 * g:32 * g + ACT_D, g, :],
                    in_=ls_ps[n][32 * g:32 * g + ACT_D, :],
                )
            for g in range(1, 4):
                nc.sync.dma_start(
                    out=red[0:ACT_D, g, :], in_=red[32 * g:32 * g + ACT_D, g, :]
                )
            nc.vector.tensor_tensor(
                out=red[0:ACT_D, 0, :], in0=red[0:ACT_D, 0, :],
                in1=red[0:ACT_D, 1, :], op=Alu.add,
            )
            nc.vector.tensor_tensor(
                out=red[0:ACT_D, 2, :], in0=red[0:ACT_D, 2, :],
                in1=red[0:ACT_D, 3, :], op=Alu.add,
            )
            nc.vector.tensor_tensor(
                out=red[0:ACT_D, 0, :], in0=red[0:ACT_D, 0, :],
                in1=red[0:ACT_D, 2, :], op=Alu.add,
            )
            nc.vector.tensor_scalar(
                out=ols[0:ACT_D, 512 * n:512 * (n + 1)], in0=red[0:ACT_D, 0, :],
                scalar1=blsb_sb[0:ACT_D, 0:1], scalar2=None, op0=Alu.add,
            )
        nc.gpsimd.dma_start(
            out=out_pm[ACT_D:2 * ACT_D, 44 * BC:45 * BC], in_=ols[0:ACT_D, :]
        )

    return nc


def _get_nc():
    if "nc" not in _cache:
        _cache["nc"] = _build_nc()
    return _cache["nc"]


# ---------------------------------------------------------------- host code
def kernel(obs, hidden0, prev_mean, prev_logstd, W_mean, b_mean,
           W_logstd, b_logstd):
    from concourse.bass_utils import run_bass_kernel_spmd
    import ml_dtypes

    mnp = {"bf16": ml_dtypes.bfloat16, "f32r": np.float32,
           "f32": np.float32}[_KDT]

    obs = np.asarray(obs, np.float32)
    h0 = np.asarray(hidden0, np.float32)
    prev_mean = np.asarray(prev_mean, np.float32)
    prev_logstd = np.asarray(prev_logstd, np.float32)
    Wm = np.asarray(W_mean, np.float32)
    b_mean = np.asarray(b_mean, np.float32)
    Wls = np.asarray(W_logstd, np.float32)
    b_logstd = np.asarray(b_logstd, np.float32)

    # shared (replicated) weight prep
    wt0 = np.ascontiguousarray(Wm[0:HID, 0:OBS_D].T).astype(mnp)      # [21, 512]
    wtm = np.concatenate(
        [
            np.ascontiguousarray(
                Wm[HID * i:HID * (i + 1),
                   OBS_D + HID * (i - 1):OBS_D + HID * i].T
            )
            for i in range(1, 11)
        ],
        axis=0,
    ).astype(mnp)                                                      # [5120, 512]
    # partition-major: [128, 40*512]
    wtmpm = np.ascontiguousarray(
        wtm.reshape(40, P, HID).transpose(1, 0, 2).reshape(P, 40 * HID)
    )
    wt11 = np.ascontiguousarray(Wm[OVERALL:OUT_DIM, IN_DIM - HID:IN_DIM].T).astype(mnp)
    wls = np.ascontiguousarray(Wls.T).astype(mnp)                      # [5653, 8]
    bm = np.zeros((45 * P, 1), np.float32)
    bm[:OUT_DIM, 0] = b_mean
    blsb = np.ascontiguousarray(b_logstd.reshape(ACT_D, 1), np.float32)

    in_maps = []
    for c in range(NCORES):
        sl = slice(c * BC, (c + 1) * BC)
        in_maps.append({
            "obsT": np.ascontiguousarray(obs[sl].T).astype(mnp),
            "h0pm": np.ascontiguousarray(
                h0[sl].T.reshape(44, P, BC).transpose(1, 0, 2)
            ).reshape(P, 44 * BC).astype(mnp),
            "wt0": wt0, "wtmpm": wtmpm, "wt11": wt11, "wls": wls,
            "bm": bm, "blsb": blsb,
            "pls": np.ascontiguousarray(prev_logstd[sl]).reshape(P, 64),
        })

    nc = _get_nc()
    _cache["last_in_maps"] = in_maps
    res = run_bass_kernel_spmd(nc, in_maps, core_ids=list(range(NCORES)))

    new_hidden = np.empty((B, OVERALL), np.float32)
    new_mean = np.empty((B, ACT_D), np.float32)
    new_lstd = np.empty((B, ACT_D), np.float32)
    log_std = np.empty((B, ACT_D), np.float32)
    for c in range(NCORES):
        sl = slice(c * BC, (c + 1) * BC)
        o = res.results[c]["out_pm"].astype(np.float32).reshape(P, 45, BC)
        # feature f = 128*t + p lives at o[p, t, n]
        new_hidden[sl] = o[:, 0:44, :].transpose(2, 1, 0).reshape(BC, OVERALL)
        new_mean[sl] = o[0:ACT_D, 44, :].T
        new_lstd[sl] = o[ACT_D:2 * ACT_D, 44, :].T
        sm = res.results[c]["out_sm"]
        log_std[sl] = sm.reshape(BC, ACT_D)

    return (prev_mean, log_std, new_hidden, new_mean, new_lstd)


# revision 41
# speedup vs baseline: 1.2537x; 1.0670x over previous
"""Trainium2 Bass kernel for nn_ActorSlowInParallel.

The reference computes, for x = [obs | hidden0] ([B, 5653]):
    new_x      = x @ W_mean.T + b_mean          [B, 5640]
    new_hidden = relu(new_x[:, :5632])
    new_mean   = new_x[:, 5632:]
    new_lstd   = x @ W_logstd.T + b_logstd      [B, 8]
    log_std    = -5 + 3.5 * (tanh(prev_logstd) + 1)
returns (prev_mean, log_std, new_hidden, new_mean, new_lstd).

W_mean is block-banded (12 staircase blocks): block 0 maps obs[21] ->
rows 0:512, blocks 1..10 map hidden0 chunk (i-1) -> rows 512i:512i+512,
block 11 maps hidden0 chunk 10 -> rows 5632:5640.  Only ~8% of the dense
matrix is nonzero, so we do 12 block matmuls instead of one dense one.

Distribution: data-parallel over batch.  Each of the 8 cores gets 1024
batch rows; weights are replicated.  No collectives.  Activations are
fed feature-major ([in, batch], transposed on host) so the contraction
dim lands on SBUF partitions; outputs are produced feature-major and
transposed back on host.
"""

import os
import sys

sys.path.insert(0, "/opt/trn_rl_repo")

import numpy as np

# ---------------------------------------------------------------- consts
P = 128
B = 8192
NCORES = 8
BC = B // NCORES            # 1024 batch rows per core
OBS_D = 21
HID = 512
OVERALL = 5632              # 11 * 512
IN_DIM = 5653
OUT_DIM = 5640
ACT_D = 8
NBLK = 11                   # h0-consuming blocks (1..11); block 11 is the mean head
NT = BC // 512              # moving tiles per batch (2)
NJ = BC // P                # 128-wide batch tiles (8)

_KDT = os.environ.get("K_DTYPE", "bf16")

_cache = {}


# ------------------------------------------------------- walrus workaround
def _patch_drain_waits():
    """The nix walrus build rejects >1 sync wait per instruction.  Tile
    attaches one wait per producer processor.  Spill the excess onto
    same-engine nops emitted directly before each instruction."""
    from concourse import tile, mybir

    if getattr(tile.TileContext, "_drain_waits_patched", False):
        return
    orig = tile.TileContext._drain_and_barrier
    MAXW = 1

    orig_lower = tile.TileContext._lower_ordered_insts

    def patched_lower(self, ordered):
        nc = self.nc
        for bb_name in list(ordered.keys()):
            new = []
            for inst in ordered[bb_name]:
                si = getattr(inst, "sync_info", None)
                if si is not None and si.on_wait and len(si.on_wait) > MAXW:
                    waits = list(si.on_wait)
                    extra, keep = waits[:-MAXW], waits[-MAXW:]
                    for i in range(0, len(extra), MAXW):
                        nop = mybir.InstNoOp(
                            name=nc.get_next_instruction_name(),
                            sync_info=mybir.SyncInfo(
                                on_wait=extra[i:i + MAXW], on_update=[]
                            ),
                            bass_nofuse=True,
                            engine=inst.engine,
                        )
                        new.append(nop)
                    inst.sync_info = mybir.SyncInfo(
                        on_wait=keep, on_update=list(si.on_update)
                    )
                new.append(inst)
            ordered[bb_name] = new
        return orig_lower(self, ordered)

    tile.TileContext._lower_ordered_insts = patched_lower

    def patched(self, tick_clock, wait_clock):
        nc = self.nc
        spill = [nc.sync.nop(nofuse=True) for _ in range(32)]
        orig(self, tick_clock, wait_clock)
        bb = None
        for func in nc.m.functions:
            for block in func.blocks:
                if any(i.name == spill[0].ins.name for i in block.instructions):
                    bb = block
                    break
        assert bb is not None
        drain = None
        seen = False
        for ins in bb.instructions:
            if ins.name == spill[0].ins.name:
                seen = True
            if seen and isinstance(ins, mybir.InstDrain):
                drain = ins
                break
        assert drain is not None
        waits = list(drain.sync_info.on_wait) if drain.sync_info else []
        if len(waits) > MAXW:
            excess = waits[MAXW:]
            drain.sync_info = mybir.SyncInfo(
                on_wait=waits[:MAXW], on_update=list(drain.sync_info.on_update)
            )
            assert len(excess) <= len(spill) * MAXW
            for i, nop in enumerate(spill):
                chunk = excess[i * MAXW:(i + 1) * MAXW]
                if not chunk:
                    break
                nop.ins.sync_info = mybir.SyncInfo(on_wait=chunk, on_update=[])

    tile.TileContext._drain_and_barrier = patched
    tile.TileContext._drain_waits_patched = True


# ------------------------------------------------------------- device code
def _build_nc():
    import concourse.bass as bass
    import concourse.mybir as mybir
    from concourse.tile import TileContext
    from contextlib import ExitStack

    _patch_drain_waits()

    f32 = mybir.dt.float32
    Alu = mybir.AluOpType
    Af = mybir.ActivationFunctionType

    mdt = {"bf16": mybir.dt.bfloat16, "f32r": mybir.dt.float32r,
           "f32": f32}[_KDT]

    def mm_ap(ap):
        return ap

    nc = bass.Bass()

    obsT = nc.declare_dram_parameter("obsT", [OBS_D, BC], mdt, isOutput=False)
    # partition-major: h0pm[p, 1024*k + n] = hidden0_shard[n, 128*k + p]
    h0pm = nc.declare_dram_parameter("h0pm", [P, 44 * BC], mdt, isOutput=False)
    wt0 = nc.declare_dram_parameter("wt0", [OBS_D, HID], mdt, isOutput=False)
    # partition-major: wtmpm[p, 512*q + m] = W_block(q//4).T[128*(q%4) + p, m]
    wtmpm = nc.declare_dram_parameter("wtmpm", [P, 40 * HID], mdt, isOutput=False)
    wt11 = nc.declare_dram_parameter("wt11", [HID, ACT_D], mdt, isOutput=False)
    wls = nc.declare_dram_parameter("wls", [IN_DIM, ACT_D], mdt, isOutput=False)
    bm = nc.declare_dram_parameter("bm", [45 * P, 1], f32, isOutput=False)
    blsb = nc.declare_dram_parameter("blsb", [ACT_D, 1], f32, isOutput=False)
    smat = nc.declare_dram_parameter("smat", [104, ACT_D], mdt, isOutput=False)
    pls = nc.declare_dram_parameter("pls", [P, 64], f32, isOutput=False)

    # partition-major output: out_pm[p, 1024*t + n] = new_x.T[128*t + p, n]
    # (t = 0..43 hidden m-tiles; t = 44: partitions 0:8 new_mean, 8:16 new_lstd)
    out_pm = nc.declare_dram_parameter("out_pm", [P, 45 * BC], mdt, isOutput=True)
    out_sm = nc.declare_dram_parameter("out_sm", [P, 64], f32, isOutput=True)

    with TileContext(nc) as tc, ExitStack() as ctx:
        misc = ctx.enter_context(tc.tile_pool(name="misc", bufs=1))
        xp = ctx.enter_context(tc.tile_pool(name="xp", bufs=4))
        outp = ctx.enter_context(tc.tile_pool(name="outp", bufs=4))
        psum = ctx.enter_context(tc.tile_pool(name="psum", bufs=5, space="PSUM"))
        psls = ctx.enter_context(tc.tile_pool(name="psls", bufs=1, space="PSUM"))

        # ---- input staging: per-block, chunk-granular tiles so the PE can
        # start on the first arriving 512KB and deps stay fine-grained
        x_tiles = {}

        def load_x(i):
            if i in x_tiles or i > NBLK:
                return
            c = i - 1
            if False:
                pass
            else:
                t = xp.tile([P, 4, BC], mdt, tag="x", name=f"x{i}")
                nc.sync.dma_start(
                    out=t[:, :, :],
                    in_=h0pm[:, 4 * BC * c:4 * BC * i].rearrange(
                        "p (k n) -> p k n", n=BC
                    ),
                )
                x_tiles[i] = [t[:, k, :] for k in range(4)]

        w_tiles = {}

        def load_w(i):
            if i in w_tiles or i > 10:
                return
            c = i - 1
            t = misc.tile([P, 4, HID], mdt, tag=f"wb{i}", name=f"wb{i}")
            nc.sync.dma_start(
                out=t[:, :, :],
                in_=wtmpm[:, 4 * HID * c:4 * HID * i].rearrange(
                    "p (k m) -> p k m", m=HID
                ),
            )
            w_tiles[i] = [t[:, k, :] for k in range(4)]

        load_x(1)
        load_w(1)

        def x_of(i):
            return x_tiles.pop(i)

        def w_of(i):
            return w_tiles.pop(i)

        # ---- resident small tensors
        obs_sb = misc.tile([P, BC], mdt, tag="obs")
        nc.sync.dma_start(out=obs_sb[0:OBS_D, :], in_=obsT[:, :])
        wt0_sb = misc.tile([P, HID], mdt, tag="wt0")
        nc.sync.dma_start(out=wt0_sb[0:OBS_D, :], in_=wt0[:, :])
        wt11_sb = misc.tile([P, 4, ACT_D], mdt, tag="wt11")
        nc.sync.dma_start(
            out=wt11_sb[:, :, :],
            in_=wt11[:, :].rearrange("(k p) e -> p k e", p=P),
        )
        wls_obs = misc.tile([P, ACT_D], mdt, tag="wlso")
        nc.sync.dma_start(out=wls_obs[0:OBS_D, :], in_=wls[0:OBS_D, :])
        wls_h0 = misc.tile([P, 44, ACT_D], mdt, tag="wlsh")
        nc.sync.dma_start(
            out=wls_h0[:, :, :],
            in_=wls[OBS_D:, :].rearrange("(k p) e -> p k e", p=P),
        )
        bm_sb = misc.tile([P, 45, 1], f32, tag="bm")
        nc.sync.dma_start(
            out=bm_sb[:, :, :], in_=bm[:, :].rearrange("(t p) o -> p t o", p=P)
        )
        blsb_sb = misc.tile([P, 1], f32, tag="blsb")
        nc.sync.dma_start(out=blsb_sb[0:ACT_D, :], in_=blsb[:, :])
        smat_sb = misc.tile([P, ACT_D], mdt, tag="smat")
        nc.sync.dma_start(out=smat_sb[0:104, :], in_=smat[:, :])
        pls_sb = misc.tile([P, 64], f32, tag="pls")
        nc.sync.dma_start(out=pls_sb[:, :], in_=pls[:, :])
        sm_sb = misc.tile([P, 64], f32, tag="sm")
        tanh_sb = misc.tile([P, 64], f32, tag="tanh")

        # ---- log_std transform of prev_logstd (elementwise)
        nc.scalar.activation(tanh_sb[:, :], pls_sb[:, :], Af.Tanh)
        nc.vector.tensor_scalar(
            out=sm_sb[:, :], in0=tanh_sb[:, :],
            scalar1=3.5, scalar2=-1.5, op0=Alu.mult, op1=Alu.add,
        )
        nc.sync.dma_start(out=out_sm[:, :], in_=sm_sb[:, :])

        # ---- logstd accumulators: [8, 512] per batch half x 4 column strips.
        # Zero each bank with a dummy start=True matmul covering all 128
        # partitions so every strip can then accumulate with start=False
        # (strip-local start=True would poison the other strips' zero region).
        ls_ps = [psls.tile([P, 512], f32, tag=f"lsps{n}", name=f"lsps{n}") for n in range(NT)]
        zmm = misc.tile([1, 512], mdt, tag="zmm")
        nc.vector.memset(zmm[:, :], 0.0)
        for n in range(NT):
            nc.tensor.matmul(
                ls_ps[n][0:P, :],
                mm_ap(zmm[0:1, 0:P]),
                mm_ap(zmm[0:1, 0:512]),
                start=True, stop=False,
            )

        # obs chunk of the logstd matmul (K=21), column strip 0
        for n in range(NT):
            nc.tensor.matmul(
                ls_ps[n][0:ACT_D, :],
                mm_ap(wls_obs[0:OBS_D, :]),
                mm_ap(obs_sb[0:OBS_D, 512 * n:512 * (n + 1)]),
                start=False, stop=False,
                tile_position=(0, 0),
            )

        ep_cnt = [0]

        def epilogue_half(ps_ap, dst_row, bias, relu, n):
            """psum [rows,512] + bias (per-partition) [+ relu] -> sbuf half."""
            rows = ps_ap.partition_size()
            ep_cnt[0] += 1
            dst = dst_row[0:rows, 512 * n:512 * (n + 1)]
            if relu:
                if ep_cnt[0] % 2 == 0:
                    nc.vector.tensor_scalar(
                        out=dst, in0=ps_ap, scalar1=bias,
                        scalar2=0.0, op0=Alu.add, op1=Alu.max,
                    )
                else:
                    nc.scalar.activation(dst, ps_ap, Af.Relu, bias=bias)
            else:
                nc.vector.tensor_scalar(
                    out=dst, in0=ps_ap, scalar1=bias,
                    scalar2=None, op0=Alu.add,
                )

        # ---- block 0: obs -> out rows 0:512
        o_sb0 = outp.tile([P, 4, BC], mdt, tag="osb", name="osb0")
        for m in range(4):
            for n in range(NT):
                ps = psum.tile([P, 512], f32, tag="ps")
                nc.tensor.matmul(
                    ps[:, :],
                    mm_ap(wt0_sb[0:OBS_D, P * m:P * (m + 1)]),
                    mm_ap(obs_sb[0:OBS_D, 512 * n:512 * (n + 1)]),
                    start=True, stop=True,
                )
                epilogue_half(ps[:, :], o_sb0[:, m, :], bm_sb[0:P, m, 0:1], True, n)
        nc.gpsimd.dma_start(
            out=out_pm[:, 0:4 * BC].rearrange("p (m n) -> p m n", n=BC),
            in_=o_sb0[:, :, :],
        )

        # ---- blocks 1..11 over hidden0 chunks
        for i in range(1, NBLK + 1):
            c = i - 1  # h0 chunk index
            xt = x_of(i)
            if i <= 10:
                w3 = w_of(i)
                o_sb = outp.tile([P, 4, BC], mdt, tag="osb", name=f"osb{i}")
                if False:
                    pass
                else:
                    for m in range(4):
                        pss = [psum.tile([P, 512], f32, tag="ps", name=f"ps{i}_{m}_{n2}") for n2 in range(NT)]
                        for k in range(4):
                            for n in range(NT):
                                nc.tensor.matmul(
                                    pss[n][:, :],
                                    mm_ap(w3[k][:, P * m:P * (m + 1)]),
                                    mm_ap(xt[k][:, 512 * n:512 * (n + 1)]),
                                    start=(k == 0), stop=(k == 3),
                                )
                        for n in range(NT):
                            epilogue_half(
                                pss[n][:, :], o_sb[:, m, :],
                                bm_sb[0:P, 4 * i + m, 0:1], True, n,
                            )
                nc.gpsimd.dma_start(
                    out=out_pm[:, 4 * BC * i:4 * BC * (i + 1)].rearrange(
                        "p (m n) -> p m n", n=BC
                    ),
                    in_=o_sb[:, :, :],
                )
            else:
                # mean head: 8 output rows, no relu
                o_sb = outp.tile([P, BC], mdt, tag="osbm", name="osbm")
                pss = [psum.tile([P, 512], f32, tag="ps", name=f"psmh_{n2}") for n2 in range(NT)]
                for k in range(4):
                    for n in range(NT):
                        nc.tensor.matmul(
                            pss[n][0:ACT_D, :],
                            mm_ap(wt11_sb[:, k, :]),
                            mm_ap(xt[k][:, 512 * n:512 * (n + 1)]),
                            start=(k == 0), stop=(k == 3),
                        )
                for n in range(NT):
                    epilogue_half(
                        pss[n][0:ACT_D, :], o_sb, bm_sb[0:ACT_D, 44, 0:1], False, n
                    )
                nc.gpsimd.dma_start(
                    out=out_pm[0:ACT_D, 44 * BC:45 * BC], in_=o_sb[0:ACT_D, :]
                )

            # logstd accumulation: strip k of the PE array accumulates every
            # (4c+k)-th chunk at psum partitions 32k:32k+8 -> 4 concurrent
            # M=8 matmuls via column tiling
            for n in range(NT):
                for k in range(4):
                    nc.tensor.matmul(
                        ls_ps[n][32 * k:32 * k + ACT_D, :],
                        mm_ap(wls_h0[:, 4 * c + k, :]),
                        mm_ap(xt[k][:, 512 * n:512 * (n + 1)]),
                        start=False,
                        stop=(i == NBLK),
                        tile_position=(0, 32 * k),
                    )
            load_x(i + 1)
            load_w(i + 1)
            load_x(i + 2)

        # ---- new_log_std epilogue: reduce the 4 strips, add bias
        ols = outp.tile([P, BC], mdt, tag="osbm", name="ols")
        # strips live at psum partitions {0,32,64,96}+0:8; reduce them with a
        # selection-matrix matmul on the PE (smat row 104 carries b_logstd
        # against a ones-row, folding the bias in for free)
        for n in range(NT):
            red = misc.tile([P, 512], mdt, tag=f"red{n}", name=f"red{n}")
            if n == 0:
                nc.vector.tensor_copy(out=red[0:104, :], in_=ls_ps[n][0:104, :])
            else:
                nc.scalar.copy(red[0:104, :], ls_ps[n][0:104, :])
            lsr = psum.tile([P, 512], f32, tag="ps", name=f"lsred{n}")
            nc.tensor.matmul(
                lsr[0:ACT_D, :],
                mm_ap(smat_sb[0:104, :]),
                mm_ap(red[0:104, :]),
                start=True, stop=True,
            )
            if n == 0:
                nc.scalar.activation(
                    ols[0:ACT_D, 512 * n:512 * (n + 1)], lsr[0:ACT_D, :],
                    Af.Identity, bias=blsb_sb[0:ACT_D, 0:1],
                )
            else:
                nc.vector.tensor_scalar(
                    out=ols[0:ACT_D, 512 * n:512 * (n + 1)], in0=lsr[0:ACT_D, :],
                    scalar1=blsb_sb[0:ACT_D, 0:1], scalar2=None, op0=Alu.add,
                )
        nc.gpsimd.dma_start(
            out=out_pm[ACT_D:2 * ACT_D, 44 * BC:45 * BC], in_=ols[0:ACT_D, :]
        )

    return nc


def _get_nc():
    if "nc" not in _cache:
        _cache["nc"] = _build_nc()
    return _cache["nc"]


# ---------------------------------------------------------------- host code
def kernel(obs, hidden0, prev_mean, prev_logstd, W_mean, b_mean,
           W_logstd, b_logstd):
    from concourse.bass_utils import run_bass_kernel_spmd
    import ml_dtypes

    mnp = {"bf16": ml_dtypes.bfloat16, "f32r": np.float32,
           "f32": np.float32}[_KDT]

    obs = np.asarray(obs, np.float32)
    h0 = np.asarray(hidden0, np.float32)
    prev_mean = np.asarray(prev_mean, np.float32)
    prev_logstd = np.asarray(prev_logstd, np.float32)
    Wm = np.asarray(W_mean, np.float32)
    b_mean = np.asarray(b_mean, np.float32)
    Wls = np.asarray(W_logstd, np.float32)
    b_logstd = np.asarray(b_logstd, np.float32)

    # shared (replicated) weight prep
    wt0 = np.ascontiguousarray(Wm[0:HID, 0:OBS_D].T).astype(mnp)      # [21, 512]
    wtm = np.concatenate(
        [
            np.ascontiguousarray(
                Wm[HID * i:HID * (i + 1),
                   OBS_D + HID * (i - 1):OBS_D + HID * i].T
            )
            for i in range(1, 11)
        ],
        axis=0,
    ).astype(mnp)                                                      # [5120, 512]
    # partition-major: [128, 40*512]
    wtmpm = np.ascontiguousarray(
        wtm.reshape(40, P, HID).transpose(1, 0, 2).reshape(P, 40 * HID)
    )
    wt11 = np.ascontiguousarray(Wm[OVERALL:OUT_DIM, IN_DIM - HID:IN_DIM].T).astype(mnp)
    wls = np.ascontiguousarray(Wls.T).astype(mnp)                      # [5653, 8]
    bm = np.zeros((45 * P, 1), np.float32)
    bm[:OUT_DIM, 0] = b_mean
    blsb = np.ascontiguousarray(b_logstd.reshape(ACT_D, 1), np.float32)
    smat = np.zeros((104, ACT_D), np.float32)
    for g in range(4):
        for j in range(ACT_D):
            smat[32 * g + j, j] = 1.0
    smat = smat.astype(mnp)

    in_maps = []
    for c in range(NCORES):
        sl = slice(c * BC, (c + 1) * BC)
        in_maps.append({
            "obsT": np.ascontiguousarray(obs[sl].T).astype(mnp),
            "h0pm": np.ascontiguousarray(
                h0[sl].T.reshape(44, P, BC).transpose(1, 0, 2)
            ).reshape(P, 44 * BC).astype(mnp),
            "wt0": wt0, "wtmpm": wtmpm, "wt11": wt11, "wls": wls,
            "bm": bm, "blsb": blsb, "smat": smat,
            "pls": np.ascontiguousarray(prev_logstd[sl]).reshape(P, 64),
        })

    nc = _get_nc()
    _cache["last_in_maps"] = in_maps
    res = run_bass_kernel_spmd(nc, in_maps, core_ids=list(range(NCORES)))

    new_hidden = np.empty((B, OVERALL), np.float32)
    new_mean = np.empty((B, ACT_D), np.float32)
    new_lstd = np.empty((B, ACT_D), np.float32)
    log_std = np.empty((B, ACT_D), np.float32)
    for c in range(NCORES):
        sl = slice(c * BC, (c + 1) * BC)
        o = res.results[c]["out_pm"].astype(np.float32).reshape(P, 45, BC)
        # feature f = 128*t + p lives at o[p, t, n]
        new_hidden[sl] = o[:, 0:44, :].transpose(2, 1, 0).reshape(BC, OVERALL)
        new_mean[sl] = o[0:ACT_D, 44, :].T
        new_lstd[sl] = o[ACT_D:2 * ACT_D, 44, :].T
        sm = res.results[c]["out_sm"]
        log_std[sl] = sm.reshape(BC, ACT_D)

    return (prev_mean, log_std, new_hidden, new_mean, new_lstd)
